# revision 14
# baseline (speedup 1.0000x reference)
"""Trainium2 Bass kernel for nn_D_GCN (Chebyshev-style GCN diffusion).

Reference computation (per batch b):
    x0 = X                       (T, N, F) node features
    x1 = A x0                    (diffusion over nodes)
    x2 = 2 A x1 - x0
    out = relu(stack_k(x_k) @ Theta1 + bias)     Theta row index = f*K + k

Algebraic refactoring (Theta_k := Theta1[k::3]):
    out = relu( g0 + A @ (h1 + A @ h2) )
    g0  = x0 (Theta_0 - Theta_2) + bias    [host, bf16]
    h1  = x0 Theta_1                       [host, fp8, x16]
    h2  = 2 x0 Theta_2                     [host, fp8]

Everything on device is computed TRANSPOSED (columns of the skinny
matrices on PSUM partitions) so the huge A matrix is always the
*moving* matmul operand at the fp8 free-dim maximum (rhs [128,2,512],
DoubleRow) while the skinny operand is stationary:
    pass 1:  w^T[c, m]  = h2^T A^T  (+ h1^T)      c = 128 output cols
    pass 2: out^T[c, n]  = w^T  A^T  (+ g0^T)
This doubles per-matmul MACs vs keeping A stationary (the baseline) and
moves the LDWEIGHTS cost to the small reused operand.

Sharding: 8 cores = 2 batches x (2 t-halves x 2 n-halves). A core's
c-columns are its 4 time steps x 32 output features; its n rows are its
2048-row output block. Pass 1 computes w^T for ALL 4096 m (2x
replicated within a batch across the n-halves - cheaper than any
collective on this runtime), pass 2 only the core's 2048 n columns.
w^T -> w (pass-2 stationary operand needs m on partitions) is done with
32 PE transposes via an fp8 identity, pipelined per 512-row block.

A^T is stored symmetrically slot-permuted (own n-half first on both
axes) so one resident 16 MiB fp8 tile serves pass 1 (all column blocks,
streamed&consumed in arrival order) and pass 2 (column blocks 0-3 =
own n columns; rows in the same slot order as w's chunks). The DMA
stream order IS the prefetch schedule: h2, A blocks (512 KiB pieces),
h1^T after the first block, g0^T last. Pass-2 matmuls for m-chunks of
block s issue as soon as block s is processed (s >= 3), so the PE
follows the A stream with a ~2.5 us tail after the last byte.
"""

import sys

if "/opt/trn_rl_repo" not in sys.path:
    sys.path.insert(0, "/opt/trn_rl_repo")

import numpy as np
import ml_dtypes

B, T, N, F, O = 2, 8, 4096, 32, 32
K = 3
NCORES = 8
NHALF = N // 2      # 2048 rows per output shard
C = 128             # output columns per core = 4 t-slices x 32 o
NCB = 8             # A^T column blocks of 512
CBW = 512           # column-block width
NPIECE = 4          # DMA pieces per column block (512 KiB each)
RCH = 32            # 128-row chunks of A^T
MCH = 32            # 128-row chunks of w

SCALE_A = 4096.0
SCALE_W = 16.0

_CACHE = {}


def _build_nc():
    import concourse.mybir as mybir
    import concourse.tile as tile
    from concourse import bacc, masks

    f32 = mybir.dt.float32
    bf16 = mybir.dt.bfloat16
    fp8 = mybir.dt.float8e4
    DR = mybir.MatmulPerfMode.DoubleRow

    nc = bacc.Bacc(None, num_devices=NCORES)

    # all inputs partition-major; A^T symmetrically slot-permuted
    A_d = nc.dram_tensor("A", [NCB, NPIECE, 128, RCH // NPIECE, CBW], fp8,
                         kind="ExternalInput")
    H2_d = nc.dram_tensor("H2", [128, RCH, C], fp8, kind="ExternalInput")
    H1_d = nc.dram_tensor("H1", [128, NCB, CBW], fp8, kind="ExternalInput")
    G0_d = nc.dram_tensor("G0", [128, NCB // 2, CBW], bf16,
                          kind="ExternalInput")
    OUT_d = nc.dram_tensor("OUT", [NCB // 2, 128, CBW], bf16,
                           kind="ExternalOutput")

    with tile.TileContext(nc) as tc:
        with (
            tc.tile_pool(name="big", bufs=1) as big,
            tc.tile_pool(name="stg", bufs=2) as stg,
            tc.tile_pool(name="ps", bufs=1, space="PSUM") as psp,
        ):
            A_sb = big.tile([128, NCB, RCH, CBW], fp8, name="Asb", tag="Asb")
            H2 = big.tile([128, RCH, C], fp8, name="H2s", tag="H2s")
            H1 = big.tile([128, NCB, CBW], fp8, name="H1s", tag="H1s")
            G0 = big.tile([128, NCB // 2, CBW], bf16, name="G0s", tag="G0s")
            W = big.tile([128, MCH, C], fp8, name="Ws", tag="Ws")
            OS = big.tile([128, NCB // 2, CBW], bf16, name="OSs", tag="OSs")
            ident = big.tile([128, 128], bf16, name="ident", tag="ident")

            masks.make_identity(nc, ident[:])

            # ---- one explicitly-ordered input stream on the SP ring ----
            nc.sync.dma_start(H2[:], H2_d[:])
            for pc in range(NPIECE):
                nc.sync.dma_start(
                    A_sb[:, 0, pc * 8:(pc + 1) * 8], A_d[0, pc])
            nc.sync.dma_start(H1[:], H1_d[:])
            for sb in range(1, NCB):
                for pc in range(NPIECE):
                    nc.sync.dma_start(
                        A_sb[:, sb, pc * 8:(pc + 1) * 8], A_d[sb, pc])
            nc.sync.dma_start(G0[:], G0_d[:])

            # ---- PE warm-up (HAM clock gate): dummy matmuls while the
            # first DMAs land so the real matmuls start at 2.4 GHz. Lands
            # in the psum bank pass 1 re-opens with start=True.
            warm_src = big.tile([128, 2, CBW], fp8, name="warmsrc",
                                tag="warmsrc")
            nc.gpsimd.memset(warm_src[:], 0.0)
            warm_ps = psp.tile([128, CBW], f32, name="warm", tag="bank4")
            NWARM = 16
            for wi in range(NWARM):
                nc.tensor.matmul(
                    warm_ps[:, 0:256], warm_src[:, :, 0:128],
                    warm_src[:, :, 0:256],
                    start=(wi == 0), stop=(wi == NWARM - 1), perf_mode=DR)

            # pass-2 psum banks accumulate across the whole stream
            ps2 = [psp.tile([128, CBW], f32, name=f"o{nb}", tag=f"bank{nb}")
                   for nb in range(4)]
            p2_count = [0] * 4

            def p2_mm(mp, nb):
                nc.tensor.matmul(
                    ps2[nb][:],
                    W[:, 2 * mp:2 * mp + 2],
                    A_sb[:, nb, 2 * mp:2 * mp + 2],
                    start=(p2_count[nb] == 0),
                    stop=(p2_count[nb] == MCH // 2 - 1),
                    perf_mode=DR)
                p2_count[nb] += 1

            def transp(wt, s, tb):
                # w^T -> w[m, c] 128x128 block via PE (bf16: fp8
                # PE-transpose needs out elem step 2); the psum->W copy
                # casts to fp8 for the pass-2 lhsT
                pst = psp.tile([128, 128], bf16, name=f"t{tb % 2}",
                               tag=f"bank{6 + tb % 2}")
                nc.tensor.transpose(
                    pst[:], wt[:, 128 * tb:128 * (tb + 1)], ident[:])
                nc.vector.tensor_copy(W[:, 4 * s + tb], pst[:])

            Relu = mybir.ActivationFunctionType.Relu
            with nc.named_scope("main"):
                # pend: PE work of slot s-1 (transposes + pass-2 matmuls),
                # emitted BETWEEN slot-s p1 piece groups so the strict-FIFO
                # PE queue always has runnable fillers ahead of a
                # DMA-stalled matmul; the last slot then leaves almost no
                # post-stream backlog.
                pend = []
                for s in range(NCB):
                    ps1 = psp.tile([128, CBW], f32, name=f"y{s % 2}",
                                   tag=f"bank{4 + s % 2}")
                    for pc in range(NPIECE):
                        for lp in range(4 * pc, 4 * pc + 4):
                            nc.tensor.matmul(
                                ps1[:], H2[:, 2 * lp:2 * lp + 2],
                                A_sb[:, s, 2 * lp:2 * lp + 2],
                                start=(lp == 0), stop=(lp == RCH // 2 - 1),
                                perf_mode=DR)
                        # drain fillers only from piece 1 on: a transpose
                        # queued at piece 0 would still be waiting on the
                        # previous slot's STT and stall the PE FIFO
                        if pc > 0:
                            take = -(-len(pend) // (NPIECE - pc))
                            for op in pend[:take]:
                                op()
                            pend = pend[take:]
                    # w^T = psum*(16/4096) + h1^T*16, in 4 column pieces
                    # so each transpose waits only on its own piece
                    wt = stg.tile([128, CBW], bf16, name="wt", tag="wt")
                    for tb in range(4):
                        cs = slice(128 * tb, 128 * (tb + 1))
                        nc.vector.scalar_tensor_tensor(
                            wt[:, cs], ps1[:, cs], 1.0 / 256.0,
                            H1[:, s, cs],
                            mybir.AluOpType.mult, mybir.AluOpType.add)
                    if s == NCB - 1:
                        break
                    # build slot-s leftovers: 4 transposes + every pass-2
                    # matmul whose inputs (rhs block nb, w chunks) now exist
                    pend = [lambda wt=wt, s=s, tb=tb: transp(wt, s, tb)
                            for tb in range(4)]
                    for mp in range(2 * s, 2 * s + 2):
                        for nb in range(min(s + 1, 4)):
                            pend.append(lambda mp=mp, nb=nb: p2_mm(mp, nb))
                    if s < 4:
                        for mp in range(2 * s):
                            pend.append(lambda mp=mp, s=s: p2_mm(mp, s))
                # tail: last slot's transposes, then nb-major pass-2 pairs
                # with each bank's drain chain right behind its stop
                s = NCB - 1
                for tb in range(4):
                    transp(wt, s, tb)
                for nb in range(4):
                    p2_mm(2 * s, nb)
                    p2_mm(2 * s + 1, nb)
                    # out^T = psum/(SCALE_A*SCALE_W) + g0^T, relu
                    nc.vector.scalar_tensor_tensor(
                        OS[:, nb], ps2[nb][:], 1.0 / 65536.0, G0[:, nb],
                        mybir.AluOpType.mult, mybir.AluOpType.add)
                    nc.scalar.activation(OS[:, nb], OS[:, nb], Relu)
                    nc.scalar.dma_start(OUT_d[nb], OS[:, nb])

    nc.compile()
    return nc


def _get_nc():
    if "nc" not in _CACHE:
        _CACHE["nc"] = _build_nc()
    return _CACHE["nc"]


def _prepare_in_maps(X, A_q, Theta1, bias):
    fp8 = ml_dtypes.float8_e4m3
    bf16 = ml_dtypes.bfloat16
    X = np.asarray(X, dtype=np.float32)
    A_q = np.asarray(A_q, dtype=np.float32)
    Theta1 = np.asarray(Theta1, dtype=np.float32)
    bias = np.asarray(bias, dtype=np.float32)

    Th = Theta1.reshape(F, K, O)
    Th0, Th1, Th2 = Th[:, 0], Th[:, 1], Th[:, 2]

    # 4 unique permuted A^T tiles (batch x n-half), shared by t-halves
    A_tiles = {}
    for b in range(B):
        At = (A_q[b].T * SCALE_A).astype(fp8)        # [l/m, m/n]
        for h in range(2):
            if h == 1:
                Ats = np.empty_like(At)
                Ats[:NHALF, :NHALF] = At[NHALF:, NHALF:]
                Ats[:NHALF, NHALF:] = At[NHALF:, :NHALF]
                Ats[NHALF:, :NHALF] = At[:NHALF, NHALF:]
                Ats[NHALF:, NHALF:] = At[:NHALF, :NHALF]
            else:
                Ats = At
            # [row, col] -> [cb, piece, p, rc', q]
            A_tiles[b, h] = np.ascontiguousarray(
                Ats.reshape(RCH, 128, NCB, CBW)
                .transpose(2, 0, 1, 3)                 # [cb, rc, p, q]
                .reshape(NCB, NPIECE, RCH // NPIECE, 128, CBW)
                .transpose(0, 1, 3, 2, 4))             # [cb, pc, p, rc', q]

    in_maps = []
    for core in range(NCORES):
        b, th, h = core // 4, (core // 2) % 2, core % 2
        Xb = X[b, 4 * th:4 * th + 4]                   # (4, N, F)
        sig = np.r_[np.arange(NHALF * h, NHALF * (h + 1)),
                    np.arange(0, NHALF * h), np.arange(NHALF * (h + 1), N)]
        # skinny mats, c = 32*t_rel + o on the trailing axis -> (N, 128)
        h2 = np.transpose(2.0 * (Xb @ Th2), (1, 0, 2)).reshape(N, C)[sig]
        h1 = np.transpose(Xb @ Th1, (1, 0, 2)).reshape(N, C)[sig]
        g0 = (np.transpose(Xb @ (Th0 - Th2), (1, 0, 2)).reshape(N, C)
              + np.tile(bias, 4)[np.newaxis, :])[NHALF * h:NHALF * (h + 1)]
        in_maps.append({
            "A": A_tiles[b, h],
            "H2": np.ascontiguousarray(
                h2.reshape(RCH, 128, C).transpose(1, 0, 2)).astype(fp8),
            "H1": np.ascontiguousarray(
                (SCALE_W * h1).reshape(NCB, CBW, C)
                .transpose(2, 0, 1)).astype(fp8),
            "G0": np.ascontiguousarray(
                g0.reshape(NCB // 2, CBW, C).transpose(2, 0, 1)).astype(bf16),
        })
    return in_maps


def run_with_results(inputs, **spmd_kwargs):
    """Returns (full_output, BassKernelResults). spmd_kwargs forwarded to
    run_bass_kernel_spmd (e.g. trace=True)."""
    from concourse.bass_utils import run_bass_kernel_spmd

    nc = _get_nc()
    in_maps = _prepare_in_maps(**inputs)
    res = run_bass_kernel_spmd(
        nc, in_maps, core_ids=list(range(NCORES)), **spmd_kwargs)

    out = np.empty((B, T, N, O), dtype=np.float32)
    for core in range(NCORES):
        b, th, h = core // 4, (core // 2) % 2, core % 2
        blk = np.asarray(res.results[core]["OUT"], dtype=np.float32)
        # [nb, p, q] -> [p, nb*q] -> [t_rel, o, n_local] -> [t, n, o]
        arr = blk.transpose(1, 0, 2).reshape(4, O, NHALF)
        out[b, 4 * th:4 * th + 4, NHALF * h:NHALF * (h + 1), :] = (
            arr.transpose(0, 2, 1))
    return out, res


def kernel(X, A_q, Theta1, bias):
    out, _ = run_with_results(
        {"X": X, "A_q": A_q, "Theta1": Theta1, "bias": bias})
    return out


# revision 15
# speedup vs baseline: 1.0278x; 1.0278x over previous
"""Trainium2 Bass kernel for nn_D_GCN (Chebyshev-style GCN diffusion).

Reference computation (per batch b):
    x0 = X                       (T, N, F) node features
    x1 = A x0                    (diffusion over nodes)
    x2 = 2 A x1 - x0
    out = relu(stack_k(x_k) @ Theta1 + bias)     Theta row index = f*K + k

Algebraic refactoring (Theta_k := Theta1[k::3]):
    out = relu( x0 (Theta_0 - Theta_2) + bias + A @ (h1 + A @ h2) )
    h1  = x0 Theta_1    [host, fp8, x16]
    h2  = 2 x0 Theta_2  [host, fp8]

Everything on device is computed TRANSPOSED (columns of the skinny
matrices on PSUM partitions) so the huge A matrix is always the
*moving* matmul operand at the fp8 free-dim maximum (rhs [128,2,512],
DoubleRow) while the skinny operand is stationary:
    pass 1:  w^T[c, m]  = h2^T A^T  (+ h1^T)      c = 128 output cols
    pass 2: out^T[c, n]  = w^T  A^T  (+ g0^T)
This doubles per-matmul MACs vs keeping A stationary and moves the
LDWEIGHTS cost to the small reused operand. The zeroth-order term
g0 = x0(Th0-Th2) is folded into the pass-2 PSUM group as one extra
block-diagonal bf16 matmul (lhsT = kron(I4, 65536*(Th0-Th2)), rhs =
x0^T), and the bias rides the final activation's per-partition bias
port - so each output bank drains as a single relu-activation + DMA.

Sharding: 8 cores = 2 batches x (2 t-halves x 2 n-halves). A core's
c-columns are its 4 time steps x 32 output features; its n rows are its
2048-row output block. Pass 1 computes w^T for ALL 4096 m (2x
replicated within a batch across the n-halves - cheaper than any
collective on this runtime), pass 2 only the core's 2048 n columns.
w^T -> w (pass-2 stationary operand needs m on partitions) is done with
PE transposes via a bf16 identity, pipelined per block.

A^T is stored symmetrically slot-permuted (own n-half first on both
axes) so one resident 16 MiB fp8 tile serves pass 1 (all column blocks,
streamed & consumed in arrival order) and pass 2 (column blocks 0-3 =
own n columns; rows in the same slot order as w's chunks). The DMA
stream order IS the prefetch schedule. The trailing column block is
split into two 256-wide halves so the unavoidable serial chain behind
the last bytes (p1 -> scale/add -> transpose -> p2 -> drain) is half
length. Transposes and pass-2 matmuls of slot s-1 are interleaved
between slot-s piece-gated matmul groups so the strict-FIFO PE queue
always has runnable work while the stream is the pacer.
"""

import sys

if "/opt/trn_rl_repo" not in sys.path:
    sys.path.insert(0, "/opt/trn_rl_repo")

import numpy as np
import ml_dtypes

B, T, N, F, O = 2, 8, 4096, 32, 32
K = 3
NCORES = 8
NHALF = N // 2      # 2048 rows per output shard
C = 128             # output columns per core = 4 t-slices x 32 o
NCB = 7             # full-width A^T column blocks of 512 (+ tapered tail)
CBW = 512
NPIECE = 4          # DMA pieces per full column block (512 KiB each)
RCH = 32            # 128-row chunks of A^T
MCH = 32            # 128-row chunks of w

SCALE_A = 4096.0
SCALE_W = 16.0
SCALE_OUT = SCALE_A * SCALE_W

_CACHE = {}


def _build_nc():
    import concourse.mybir as mybir
    import concourse.tile as tile
    from concourse import bacc, masks

    f32 = mybir.dt.float32
    bf16 = mybir.dt.bfloat16
    fp8 = mybir.dt.float8e4
    DR = mybir.MatmulPerfMode.DoubleRow

    nc = bacc.Bacc(None, num_devices=NCORES)

    # all inputs partition-major; A^T symmetrically slot-permuted
    A_d = nc.dram_tensor("A", [NCB, NPIECE, 128, RCH // NPIECE, CBW], fp8,
                         kind="ExternalInput")
    A7_d = nc.dram_tensor("A7", [2, 2, 128, RCH // 2, CBW // 2], fp8,
                          kind="ExternalInput")
    H2_d = nc.dram_tensor("H2", [128, RCH, C], fp8, kind="ExternalInput")
    H1_d = nc.dram_tensor("H1", [128, NCB + 1, CBW], fp8,
                          kind="ExternalInput")
    X0_d = nc.dram_tensor("X0", [128, 4, CBW], bf16, kind="ExternalInput")
    TK_d = nc.dram_tensor("TK", [128, C], bf16, kind="ExternalInput")
    BS_d = nc.dram_tensor("BS", [128, 1], f32, kind="ExternalInput")
    OUT_d = nc.dram_tensor("OUT", [4, 128, CBW], bf16, kind="ExternalOutput")

    with tile.TileContext(nc) as tc:
        with (
            tc.tile_pool(name="big", bufs=1) as big,
            tc.tile_pool(name="stg", bufs=2) as stg,
            tc.tile_pool(name="ps", bufs=1, space="PSUM") as psp,
        ):
            A_sb = big.tile([128, NCB, RCH, CBW], fp8, name="Asb", tag="Asb")
            A7_sb = big.tile([128, 2, RCH, CBW // 2], fp8, name="A7sb",
                             tag="A7sb")
            H2 = big.tile([128, RCH, C], fp8, name="H2s", tag="H2s")
            H1 = big.tile([128, NCB + 1, CBW], fp8, name="H1s", tag="H1s")
            X0 = big.tile([128, 4, CBW], bf16, name="X0s", tag="X0s")
            TK = big.tile([128, C], bf16, name="TKs", tag="TKs")
            BS = big.tile([128, 1], f32, name="BSs", tag="BSs")
            W = big.tile([128, MCH, C], fp8, name="Ws", tag="Ws")
            OS = big.tile([128, 4, CBW], bf16, name="OSs", tag="OSs")
            ident = big.tile([128, 128], bf16, name="ident", tag="ident")

            masks.make_identity(nc, ident[:])

            # ---- one explicitly-ordered input stream on the SP ring ----
            nc.sync.dma_start(H2[:], H2_d[:])
            for pc in range(NPIECE):
                nc.sync.dma_start(
                    A_sb[:, 0, pc * 8:(pc + 1) * 8], A_d[0, pc])
            nc.sync.dma_start(X0[:], X0_d[:])
            nc.sync.dma_start(TK[:], TK_d[:])
            nc.sync.dma_start(BS[:], BS_d[:])
            nc.sync.dma_start(H1[:], H1_d[:])
            for sb in range(1, NCB):
                for pc in range(NPIECE):
                    nc.sync.dma_start(
                        A_sb[:, sb, pc * 8:(pc + 1) * 8], A_d[sb, pc])
            for hh in range(2):
                for pc in range(2):
                    nc.sync.dma_start(
                        A7_sb[:, hh, pc * 16:(pc + 1) * 16], A7_d[hh, pc])

            # ---- PE warm-up (HAM clock gate): dummy matmuls while the
            # first DMAs land so the real matmuls start at 2.4 GHz. Lands
            # in the psum bank pass 1 re-opens with start=True.
            warm_src = big.tile([128, 2, CBW], fp8, name="warmsrc",
                                tag="warmsrc")
            nc.gpsimd.memset(warm_src[:], 0.0)
            warm_ps = psp.tile([128, CBW], f32, name="warm", tag="bank4")
            NWARM = 16
            for wi in range(NWARM):
                nc.tensor.matmul(
                    warm_ps[:, 0:256], warm_src[:, :, 0:128],
                    warm_src[:, :, 0:256],
                    start=(wi == 0), stop=(wi == NWARM - 1), perf_mode=DR)

            # pass-2 psum banks accumulate across the whole stream:
            # 1 bf16 g0 matmul + 16 fp8 DR matmuls each
            ps2 = [psp.tile([128, CBW], f32, name=f"o{nb}", tag=f"bank{nb}")
                   for nb in range(4)]
            p2_count = [0] * 4
            P2N = MCH // 2 + 1

            def p2_mm(mp, nb):
                nc.tensor.matmul(
                    ps2[nb][:],
                    W[:, 2 * mp:2 * mp + 2],
                    A_sb[:, nb, 2 * mp:2 * mp + 2],
                    start=(p2_count[nb] == 0),
                    stop=(p2_count[nb] == P2N - 1),
                    perf_mode=DR)
                p2_count[nb] += 1

            def g0_mm(nb):
                # out^T += 65536 * (Th0-Th2)^T x0^T, block-diagonal in t
                nc.tensor.matmul(
                    ps2[nb][:], TK[:], X0[:, nb],
                    start=(p2_count[nb] == 0),
                    stop=(p2_count[nb] == P2N - 1))
                p2_count[nb] += 1

            def transp(wt, mc, tb):
                # w^T -> w[m, c] 128x128 block via PE (bf16: fp8
                # PE-transpose needs out elem step 2); the psum->W copy
                # casts to fp8 for the pass-2 lhsT
                pst = psp.tile([128, 128], bf16, name=f"t{tb % 2}",
                               tag=f"bank{6 + tb % 2}")
                nc.tensor.transpose(
                    pst[:], wt[:, 128 * tb:128 * (tb + 1)], ident[:])
                nc.vector.tensor_copy(W[:, mc], pst[:])

            Relu = mybir.ActivationFunctionType.Relu

            def drain(nb):
                # out^T = relu(psum/SCALE_OUT + bias_c); DMA on the (now
                # idle) SP ring so the Scalar queue only runs the ACTs
                nc.scalar.activation(OS[:, nb], ps2[nb][:], Relu,
                                     bias=BS[:], scale=1.0 / SCALE_OUT)
                nc.sync.dma_start(OUT_d[nb], OS[:, nb])

            with nc.named_scope("main"):
                # pend: PE work of slot s-1 (transposes + pass-2 matmuls),
                # emitted BETWEEN slot-s p1 piece groups so the strict-FIFO
                # PE queue always has runnable fillers ahead of a
                # DMA-stalled matmul.
                pend = []

                def drain_pend(pc, npieces):
                    nonlocal pend
                    take = -(-len(pend) // (npieces - pc))
                    for op in pend[:take]:
                        op()
                    pend = pend[take:]

                for s in range(NCB):
                    ps1 = psp.tile([128, CBW], f32, name=f"y{s % 2}",
                                   tag=f"bank{4 + s % 2}")
                    for pc in range(NPIECE):
                        for lp in range(4 * pc, 4 * pc + 4):
                            nc.tensor.matmul(
                                ps1[:], H2[:, 2 * lp:2 * lp + 2],
                                A_sb[:, s, 2 * lp:2 * lp + 2],
                                start=(lp == 0), stop=(lp == RCH // 2 - 1),
                                perf_mode=DR)
                        drain_pend(pc, NPIECE)
                    # w^T = psum*(16/4096) + h1^T*16   [bf16 staging]
                    wt = stg.tile([128, CBW], bf16, name="wt", tag="wt")
                    nc.vector.scalar_tensor_tensor(
                        wt[:], ps1[:], 1.0 / 256.0, H1[:, s],
                        mybir.AluOpType.mult, mybir.AluOpType.add)
                    # slot-s leftovers: 4 transposes + every pass-2 matmul
                    # whose inputs (rhs block nb, w chunks) now exist
                    pend = [lambda wt=wt, s=s, tb=tb: transp(wt, 4 * s + tb,
                                                             tb)
                            for tb in range(4)]
                    if s < 4:
                        pend.append(lambda s=s: g0_mm(s))
                    for mp in range(2 * s, 2 * s + 2):
                        for nb in range(min(s + 1, 4)):
                            pend.append(lambda mp=mp, nb=nb: p2_mm(mp, nb))
                    if s < 4:
                        for mp in range(2 * s):
                            pend.append(lambda mp=mp, s=s: p2_mm(mp, s))

                # tapered tail: two 256-wide column halves, each its own
                # psum group in one bank, so the post-stream serial chain
                # (p1 -> stt -> transpose -> p2 -> drain) is half length
                ps7 = psp.tile([128, CBW], f32, name="y7", tag="bank5")
                for hh in range(2):
                    half = slice(256 * hh, 256 * (hh + 1))
                    for pc in range(2):
                        for lp in range(8 * pc, 8 * pc + 8):
                            nc.tensor.matmul(
                                ps7[:, half], H2[:, 2 * lp:2 * lp + 2],
                                A7_sb[:, hh, 2 * lp:2 * lp + 2],
                                start=(lp == 0), stop=(lp == RCH // 2 - 1),
                                perf_mode=DR)
                        drain_pend(2 * hh + pc, 4)
                    wt7 = stg.tile([128, 256], bf16, name="wt7", tag="wt7")
                    nc.vector.scalar_tensor_tensor(
                        wt7[:], ps7[:, half], 1.0 / 256.0,
                        H1[:, NCB, half],
                        mybir.AluOpType.mult, mybir.AluOpType.add)
                    for tb in range(2):
                        transp(wt7, 28 + 2 * hh + tb, tb)
                    mp = 14 + hh
                    if hh == 0:
                        for nb in range(4):
                            p2_mm(mp, nb)
                    else:
                        for nb in range(4):
                            p2_mm(mp, nb)
                            drain(nb)

    nc.compile()
    return nc


def _get_nc():
    if "nc" not in _CACHE:
        _CACHE["nc"] = _build_nc()
    return _CACHE["nc"]


def _prepare_in_maps(X, A_q, Theta1, bias):
    fp8 = ml_dtypes.float8_e4m3
    bf16 = ml_dtypes.bfloat16
    X = np.asarray(X, dtype=np.float32)
    A_q = np.asarray(A_q, dtype=np.float32)
    Theta1 = np.asarray(Theta1, dtype=np.float32)
    bias = np.asarray(bias, dtype=np.float32)

    Th = Theta1.reshape(F, K, O)
    Th0, Th1, Th2 = Th[:, 0], Th[:, 1], Th[:, 2]

    # 4 unique permuted A^T tile pairs (batch x n-half), shared by t-halves
    A_tiles = {}
    for b in range(B):
        At = (A_q[b].T * SCALE_A).astype(fp8)        # [l/m, m/n]
        for h in range(2):
            if h == 1:
                Ats = np.empty_like(At)
                Ats[:NHALF, :NHALF] = At[NHALF:, NHALF:]
                Ats[:NHALF, NHALF:] = At[NHALF:, :NHALF]
                Ats[NHALF:, :NHALF] = At[:NHALF, NHALF:]
                Ats[NHALF:, NHALF:] = At[:NHALF, :NHALF]
            else:
                Ats = At
            # full blocks: [row, col] -> [cb, piece, p, rc', q]
            main = np.ascontiguousarray(
                Ats[:, :NCB * CBW].reshape(RCH, 128, NCB, CBW)
                .transpose(2, 0, 1, 3)
                .reshape(NCB, NPIECE, RCH // NPIECE, 128, CBW)
                .transpose(0, 1, 3, 2, 4))
            # tapered tail block: two 256-col halves, 2 row pieces each
            tail = np.ascontiguousarray(
                Ats[:, NCB * CBW:].reshape(RCH, 128, 2, CBW // 2)
                .transpose(2, 0, 1, 3)
                .reshape(2, 2, RCH // 2, 128, CBW // 2)
                .transpose(0, 1, 3, 2, 4))
            A_tiles[b, h] = (main, tail)

    in_maps = []
    for core in range(NCORES):
        b, th, h = core // 4, (core // 2) % 2, core % 2
        Xb = X[b, 4 * th:4 * th + 4]                   # (4, N, F)
        sig = np.r_[np.arange(NHALF * h, NHALF * (h + 1)),
                    np.arange(0, NHALF * h), np.arange(NHALF * (h + 1), N)]
        # skinny mats, c = 32*t_rel + o on the trailing axis -> (N, 128)
        h2 = np.transpose(2.0 * (Xb @ Th2), (1, 0, 2)).reshape(N, C)[sig]
        h1 = np.transpose(Xb @ Th1, (1, 0, 2)).reshape(N, C)[sig]
        x0 = Xb[:, NHALF * h:NHALF * (h + 1)]          # (4, 2048, 32)
        main, tail = A_tiles[b, h]
        in_maps.append({
            "A": main,
            "A7": tail,
            "H2": np.ascontiguousarray(
                h2.reshape(RCH, 128, C).transpose(1, 0, 2)).astype(fp8),
            "H1": np.ascontiguousarray(
                (SCALE_W * h1).reshape(NCB + 1, CBW, C)
                .transpose(2, 0, 1)).astype(fp8),
            # x0^T with (t', f) on partitions, own n-half on free
            "X0": np.ascontiguousarray(
                x0.transpose(0, 2, 1).reshape(C, 4, CBW)).astype(bf16),
            "TK": np.ascontiguousarray(
                np.kron(np.eye(4, dtype=np.float32),
                        SCALE_OUT * (Th0 - Th2))).astype(bf16),
            "BS": np.ascontiguousarray(
                np.tile(bias, 4)[:, np.newaxis]).astype(np.float32),
        })
    return in_maps


def run_with_results(inputs, **spmd_kwargs):
    """Returns (full_output, BassKernelResults). spmd_kwargs forwarded to
    run_bass_kernel_spmd (e.g. trace=True)."""
    from concourse.bass_utils import run_bass_kernel_spmd

    nc = _get_nc()
    in_maps = _prepare_in_maps(**inputs)
    res = run_bass_kernel_spmd(
        nc, in_maps, core_ids=list(range(NCORES)), **spmd_kwargs)

    out = np.empty((B, T, N, O), dtype=np.float32)
    for core in range(NCORES):
        b, th, h = core // 4, (core // 2) % 2, core % 2
        blk = np.asarray(res.results[core]["OUT"], dtype=np.float32)
        # [nb, p, q] -> [p, nb*q] -> [t_rel, o, n_local] -> [t, n, o]
        arr = blk.transpose(1, 0, 2).reshape(4, O, NHALF)
        out[b, 4 * th:4 * th + 4, NHALF * h:NHALF * (h + 1), :] = (
            arr.transpose(0, 2, 1))
    return out, res


def kernel(X, A_q, Theta1, bias):
    out, _ = run_with_results(
        {"X": X, "A_q": A_q, "Theta1": Theta1, "bias": bias})
    return out


# revision 16
# speedup vs baseline: 1.1600x; 1.1287x over previous
"""Trainium2 Bass kernel for nn_D_GCN (Chebyshev-style GCN diffusion).

Reference computation (per batch b):
    x0 = X                       (T, N, F) node features
    x1 = A x0                    (diffusion over nodes)
    x2 = 2 A x1 - x0
    out = relu(stack_k(x_k) @ Theta1 + bias)     Theta row index = f*K + k

Algebraic refactoring (Theta_k := Theta1[k::3]):
    out = relu( x0 (Theta_0 - Theta_2) + bias + A @ (h1 + A @ h2) )
    h1  = x0 Theta_1    [host, fp8, x16]
    h2  = 2 x0 Theta_2  [host, fp8]

Everything on device is computed TRANSPOSED (columns of the skinny
matrices on PSUM partitions) so the huge A matrix is always the
*moving* matmul operand at the fp8 free-dim maximum (rhs [128,2,512],
DoubleRow) while the skinny operand is stationary:
    pass 1:  w^T[c, m]  = h2^T A^T  (+ h1^T)      c = 128 output cols
    pass 2: out^T[c, n]  = w^T  A^T  (+ g0^T)
This doubles per-matmul MACs vs keeping A stationary and moves the
LDWEIGHTS cost to the small reused operand. The zeroth-order term
g0 = x0(Th0-Th2) is folded into the pass-2 PSUM group as one extra
block-diagonal bf16 matmul (lhsT = kron(I4, 65536*(Th0-Th2)), rhs =
x0^T), and the bias rides the final activation's per-partition bias
port - so each output bank drains as a single relu-activation + DMA.

Sharding: 8 cores = 2 batches x (2 t-halves x 2 n-halves). A core's
c-columns are its 4 time steps x 32 output features; its n rows are its
2048-row output block. Pass 1 computes w^T for ALL 4096 m (2x
replicated within a batch across the n-halves - cheaper than any
collective on this runtime), pass 2 only the core's 2048 n columns.
w^T -> w (pass-2 stationary operand needs m on partitions) is done with
PE transposes via a bf16 identity, pipelined per block.

A^T is stored symmetrically slot-permuted (own n-half first on both
axes) so one resident 16 MiB fp8 tile serves pass 1 (all column blocks,
streamed & consumed in arrival order) and pass 2 (column blocks 0-3 =
own n columns; rows in the same slot order as w's chunks). The DMA
stream order IS the prefetch schedule. The trailing column block is
split into two 256-wide halves so the unavoidable serial chain behind
the last bytes (p1 -> scale/add -> transpose -> p2 -> drain) is half
length. Transposes and pass-2 matmuls of slot s-1 are interleaved
between slot-s piece-gated matmul groups so the strict-FIFO PE queue
always has runnable work while the stream is the pacer.
"""

import sys

if "/opt/trn_rl_repo" not in sys.path:
    sys.path.insert(0, "/opt/trn_rl_repo")

import numpy as np
import ml_dtypes

B, T, N, F, O = 2, 8, 4096, 32, 32
K = 3
NCORES = 8
NHALF = N // 2      # 2048 rows per output shard
C = 128             # output columns per core = 4 t-slices x 32 o
NCB = 7             # full-width A^T column blocks of 512 (+ tapered tail)
CBW = 512
NPIECE = 4          # DMA pieces per full column block (512 KiB each)
RCH = 32            # 128-row chunks of A^T
MCH = 32            # 128-row chunks of w

SCALE_A = 4096.0
SCALE_W = 16.0
SCALE_OUT = SCALE_A * SCALE_W

_CACHE = {}


def _build_nc():
    import concourse.mybir as mybir
    import concourse.tile as tile
    from concourse import bacc, masks

    f32 = mybir.dt.float32
    bf16 = mybir.dt.bfloat16
    fp8 = mybir.dt.float8e4
    DR = mybir.MatmulPerfMode.DoubleRow

    nc = bacc.Bacc(None, num_devices=NCORES)

    # all inputs partition-major; A^T symmetrically slot-permuted
    A_d = nc.dram_tensor("A", [NCB, NPIECE, 128, RCH // NPIECE, CBW], fp8,
                         kind="ExternalInput")
    A7_d = nc.dram_tensor("A7", [2, 2, 128, RCH // 2, CBW // 2], fp8,
                          kind="ExternalInput")
    H2_d = nc.dram_tensor("H2", [128, RCH, C], fp8, kind="ExternalInput")
    H1_d = nc.dram_tensor("H1", [128, NCB + 1, CBW], fp8,
                          kind="ExternalInput")
    X0_d = nc.dram_tensor("X0", [128, 4, CBW], bf16, kind="ExternalInput")
    TK_d = nc.dram_tensor("TK", [128, C], bf16, kind="ExternalInput")
    BS_d = nc.dram_tensor("BS", [128, 1], f32, kind="ExternalInput")
    OUT_d = nc.dram_tensor("OUT", [4, 128, CBW], bf16, kind="ExternalOutput")

    with tile.TileContext(nc) as tc:
        with (
            tc.tile_pool(name="big", bufs=1) as big,
            tc.tile_pool(name="stg", bufs=2) as stg,
            tc.tile_pool(name="ps", bufs=1, space="PSUM") as psp,
        ):
            A_sb = big.tile([128, NCB, RCH, CBW], fp8, name="Asb", tag="Asb")
            A7_sb = big.tile([128, 2, RCH, CBW // 2], fp8, name="A7sb",
                             tag="A7sb")
            H2 = big.tile([128, RCH, C], fp8, name="H2s", tag="H2s")
            H1 = big.tile([128, NCB + 1, CBW], fp8, name="H1s", tag="H1s")
            X0 = big.tile([128, 4, CBW], bf16, name="X0s", tag="X0s")
            TK = big.tile([128, C], bf16, name="TKs", tag="TKs")
            BS = big.tile([128, 1], f32, name="BSs", tag="BSs")
            W = big.tile([128, MCH, C], fp8, name="Ws", tag="Ws")
            OS = big.tile([128, 4, CBW], bf16, name="OSs", tag="OSs")
            ident = big.tile([128, 128], bf16, name="ident", tag="ident")

            masks.make_identity(nc, ident[:])

            # ---- one explicitly-ordered input stream on the SP ring ----
            nc.sync.dma_start(H2[:], H2_d[:])
            for pc in range(NPIECE):
                nc.sync.dma_start(
                    A_sb[:, 0, pc * 8:(pc + 1) * 8], A_d[0, pc])
            # side tensors ride the otherwise-idle ACT HWDGE ring so the
            # SP ring stays a pure stream of big transfers
            nc.scalar.dma_start(X0[:], X0_d[:])
            nc.scalar.dma_start(TK[:], TK_d[:])
            nc.scalar.dma_start(BS[:], BS_d[:])
            nc.sync.dma_start(H1[:], H1_d[:])
            for sb in range(1, NCB):
                for pc in range(NPIECE):
                    nc.sync.dma_start(
                        A_sb[:, sb, pc * 8:(pc + 1) * 8], A_d[sb, pc])
            for hh in range(2):
                for pc in range(2):
                    nc.sync.dma_start(
                        A7_sb[:, hh, pc * 16:(pc + 1) * 16], A7_d[hh, pc])

            # ---- PE warm-up (HAM clock gate): dummy matmuls while the
            # first DMAs land so the real matmuls start at 2.4 GHz. Lands
            # in the psum bank pass 1 re-opens with start=True.
            warm_src = big.tile([128, 2, CBW], fp8, name="warmsrc",
                                tag="warmsrc")
            nc.gpsimd.memset(warm_src[:], 0.0)
            warm_ps = psp.tile([128, CBW], f32, name="warm", tag="bank4")
            NWARM = 16
            for wi in range(NWARM):
                nc.tensor.matmul(
                    warm_ps[:, 0:256], warm_src[:, :, 0:128],
                    warm_src[:, :, 0:256],
                    start=(wi == 0), stop=(wi == NWARM - 1), perf_mode=DR)

            # pass-2 psum banks accumulate across the whole stream:
            # 1 bf16 g0 matmul + 16 fp8 DR matmuls each
            ps2 = [psp.tile([128, CBW], f32, name=f"o{nb}", tag=f"bank{nb}")
                   for nb in range(4)]
            p2_count = [0] * 4
            P2N = MCH // 2 + 1

            def p2_mm(mp, nb):
                nc.tensor.matmul(
                    ps2[nb][:],
                    W[:, 2 * mp:2 * mp + 2],
                    A_sb[:, nb, 2 * mp:2 * mp + 2],
                    start=(p2_count[nb] == 0),
                    stop=(p2_count[nb] == P2N - 1),
                    perf_mode=DR)
                p2_count[nb] += 1

            def g0_mm(nb):
                # out^T += 65536 * (Th0-Th2)^T x0^T, block-diagonal in t
                nc.tensor.matmul(
                    ps2[nb][:], TK[:], X0[:, nb],
                    start=(p2_count[nb] == 0),
                    stop=(p2_count[nb] == P2N - 1))
                p2_count[nb] += 1

            def transp(wt, mc, tb):
                # w^T -> w[m, c] 128x128 block via PE (bf16: fp8
                # PE-transpose needs out elem step 2); the psum->W copy
                # casts to fp8 for the pass-2 lhsT
                pst = psp.tile([128, 128], bf16, name=f"t{tb % 2}",
                               tag=f"bank{6 + tb % 2}")
                nc.tensor.transpose(
                    pst[:], wt[:, 128 * tb:128 * (tb + 1)], ident[:])
                nc.vector.tensor_copy(W[:, mc], pst[:])

            Relu = mybir.ActivationFunctionType.Relu

            def drain(nb):
                # out^T = relu(psum/SCALE_OUT + bias_c); DMA on the (now
                # idle) SP ring so the Scalar queue only runs the ACTs
                nc.scalar.activation(OS[:, nb], ps2[nb][:], Relu,
                                     bias=BS[:], scale=1.0 / SCALE_OUT)
                nc.sync.dma_start(OUT_d[nb], OS[:, nb])

            with nc.named_scope("main"):
                # pend: PE work of slot s-1 (transposes + pass-2 matmuls),
                # emitted BETWEEN slot-s p1 piece groups so the strict-FIFO
                # PE queue always has runnable fillers ahead of a
                # DMA-stalled matmul.
                pend = []

                def drain_pend(pc, npieces):
                    nonlocal pend
                    take = -(-len(pend) // (npieces - pc))
                    for op in pend[:take]:
                        op()
                    pend = pend[take:]

                for s in range(NCB):
                    ps1 = psp.tile([128, CBW], f32, name=f"y{s % 2}",
                                   tag=f"bank{4 + s % 2}")
                    for pc in range(NPIECE):
                        for lp in range(4 * pc, 4 * pc + 4):
                            nc.tensor.matmul(
                                ps1[:], H2[:, 2 * lp:2 * lp + 2],
                                A_sb[:, s, 2 * lp:2 * lp + 2],
                                start=(lp == 0), stop=(lp == RCH // 2 - 1),
                                perf_mode=DR)
                        drain_pend(pc, NPIECE)
                    # w^T = psum*(16/4096) + h1^T*16   [bf16 staging]
                    wt = stg.tile([128, CBW], bf16, name="wt", tag="wt")
                    nc.vector.scalar_tensor_tensor(
                        wt[:], ps1[:], 1.0 / 256.0, H1[:, s],
                        mybir.AluOpType.mult, mybir.AluOpType.add)
                    # slot-s leftovers: 4 transposes + every pass-2 matmul
                    # whose inputs (rhs block nb, w chunks) now exist
                    pend = [lambda wt=wt, s=s, tb=tb: transp(wt, 4 * s + tb,
                                                             tb)
                            for tb in range(4)]
                    if s < 4:
                        pend.append(lambda s=s: g0_mm(s))
                    for mp in range(2 * s, 2 * s + 2):
                        for nb in range(min(s + 1, 4)):
                            pend.append(lambda mp=mp, nb=nb: p2_mm(mp, nb))
                    if s < 4:
                        for mp in range(2 * s):
                            pend.append(lambda mp=mp, s=s: p2_mm(mp, s))

                # tapered tail: two 256-wide column halves, each its own
                # psum group in one bank, so the post-stream serial chain
                # (p1 -> stt -> transpose -> p2 -> drain) is half length
                ps7 = psp.tile([128, CBW], f32, name="y7", tag="bank5")
                for hh in range(2):
                    half = slice(256 * hh, 256 * (hh + 1))
                    for pc in range(2):
                        for lp in range(8 * pc, 8 * pc + 8):
                            nc.tensor.matmul(
                                ps7[:, half], H2[:, 2 * lp:2 * lp + 2],
                                A7_sb[:, hh, 2 * lp:2 * lp + 2],
                                start=(lp == 0), stop=(lp == RCH // 2 - 1),
                                perf_mode=DR)
                        drain_pend(2 * hh + pc, 4)
                    wt7 = stg.tile([128, 256], bf16, name="wt7", tag="wt7")
                    nc.vector.scalar_tensor_tensor(
                        wt7[:], ps7[:, half], 1.0 / 256.0,
                        H1[:, NCB, half],
                        mybir.AluOpType.mult, mybir.AluOpType.add)
                    for tb in range(2):
                        transp(wt7, 28 + 2 * hh + tb, tb)
                    mp = 14 + hh
                    if hh == 0:
                        for nb in range(4):
                            p2_mm(mp, nb)
                    else:
                        for nb in range(4):
                            p2_mm(mp, nb)
                            drain(nb)

    nc.compile()
    return nc


def _get_nc():
    if "nc" not in _CACHE:
        _CACHE["nc"] = _build_nc()
    return _CACHE["nc"]


def _prepare_in_maps(X, A_q, Theta1, bias):
    fp8 = ml_dtypes.float8_e4m3
    bf16 = ml_dtypes.bfloat16
    X = np.asarray(X, dtype=np.float32)
    A_q = np.asarray(A_q, dtype=np.float32)
    Theta1 = np.asarray(Theta1, dtype=np.float32)
    bias = np.asarray(bias, dtype=np.float32)

    Th = Theta1.reshape(F, K, O)
    Th0, Th1, Th2 = Th[:, 0], Th[:, 1], Th[:, 2]

    # 4 unique permuted A^T tile pairs (batch x n-half), shared by t-halves
    A_tiles = {}
    for b in range(B):
        At = (A_q[b].T * SCALE_A).astype(fp8)        # [l/m, m/n]
        for h in range(2):
            if h == 1:
                Ats = np.empty_like(At)
                Ats[:NHALF, :NHALF] = At[NHALF:, NHALF:]
                Ats[:NHALF, NHALF:] = At[NHALF:, :NHALF]
                Ats[NHALF:, :NHALF] = At[:NHALF, NHALF:]
                Ats[NHALF:, NHALF:] = At[:NHALF, :NHALF]
            else:
                Ats = At
            # full blocks: [row, col] -> [cb, piece, p, rc', q]
            main = np.ascontiguousarray(
                Ats[:, :NCB * CBW].reshape(RCH, 128, NCB, CBW)
                .transpose(2, 0, 1, 3)
                .reshape(NCB, NPIECE, RCH // NPIECE, 128, CBW)
                .transpose(0, 1, 3, 2, 4))
            # tapered tail block: two 256-col halves, 2 row pieces each
            tail = np.ascontiguousarray(
                Ats[:, NCB * CBW:].reshape(RCH, 128, 2, CBW // 2)
                .transpose(2, 0, 1, 3)
                .reshape(2, 2, RCH // 2, 128, CBW // 2)
                .transpose(0, 1, 3, 2, 4))
            A_tiles[b, h] = (main, tail)

    in_maps = []
    for core in range(NCORES):
        b, th, h = core // 4, (core // 2) % 2, core % 2
        Xb = X[b, 4 * th:4 * th + 4]                   # (4, N, F)
        sig = np.r_[np.arange(NHALF * h, NHALF * (h + 1)),
                    np.arange(0, NHALF * h), np.arange(NHALF * (h + 1), N)]
        # skinny mats, c = 32*t_rel + o on the trailing axis -> (N, 128)
        h2 = np.transpose(2.0 * (Xb @ Th2), (1, 0, 2)).reshape(N, C)[sig]
        h1 = np.transpose(Xb @ Th1, (1, 0, 2)).reshape(N, C)[sig]
        x0 = Xb[:, NHALF * h:NHALF * (h + 1)]          # (4, 2048, 32)
        main, tail = A_tiles[b, h]
        in_maps.append({
            "A": main,
            "A7": tail,
            "H2": np.ascontiguousarray(
                h2.reshape(RCH, 128, C).transpose(1, 0, 2)).astype(fp8),
            "H1": np.ascontiguousarray(
                (SCALE_W * h1).reshape(NCB + 1, CBW, C)
                .transpose(2, 0, 1)).astype(fp8),
            # x0^T with (t', f) on partitions, own n-half on free
            "X0": np.ascontiguousarray(
                x0.transpose(0, 2, 1).reshape(C, 4, CBW)).astype(bf16),
            "TK": np.ascontiguousarray(
                np.kron(np.eye(4, dtype=np.float32),
                        SCALE_OUT * (Th0 - Th2))).astype(bf16),
            "BS": np.ascontiguousarray(
                np.tile(bias, 4)[:, np.newaxis]).astype(np.float32),
        })
    return in_maps


def run_with_results(inputs, **spmd_kwargs):
    """Returns (full_output, BassKernelResults). spmd_kwargs forwarded to
    run_bass_kernel_spmd (e.g. trace=True)."""
    from concourse.bass_utils import run_bass_kernel_spmd

    nc = _get_nc()
    in_maps = _prepare_in_maps(**inputs)
    res = run_bass_kernel_spmd(
        nc, in_maps, core_ids=list(range(NCORES)), **spmd_kwargs)

    out = np.empty((B, T, N, O), dtype=np.float32)
    for core in range(NCORES):
        b, th, h = core // 4, (core // 2) % 2, core % 2
        blk = np.asarray(res.results[core]["OUT"], dtype=np.float32)
        # [nb, p, q] -> [p, nb*q] -> [t_rel, o, n_local] -> [t, n, o]
        arr = blk.transpose(1, 0, 2).reshape(4, O, NHALF)
        out[b, 4 * th:4 * th + 4, NHALF * h:NHALF * (h + 1), :] = (
            arr.transpose(0, 2, 1))
    return out, res


def kernel(X, A_q, Theta1, bias):
    out, _ = run_with_results(
        {"X": X, "A_q": A_q, "Theta1": Theta1, "bias": bias})
    return out


# revision 18
# speedup vs baseline: 1.2091x; 1.0423x over previous
"""Trainium2 Bass kernel for nn_D_GCN (Chebyshev-style GCN diffusion).

Reference computation (per batch b):
    x0 = X                       (T, N, F) node features
    x1 = A x0                    (diffusion over nodes)
    x2 = 2 A x1 - x0
    out = relu(stack_k(x_k) @ Theta1 + bias)     Theta row index = f*K + k

Algebraic refactoring (Theta_k := Theta1[k::3]):
    out = relu( x0 (Theta_0 - Theta_2) + bias + A @ (h1 + A @ h2) )
    h1  = x0 Theta_1    [host, fp8, x16]
    h2  = 2 x0 Theta_2  [host, fp8]

Everything on device is computed TRANSPOSED (columns of the skinny
matrices on PSUM partitions) so the huge A matrix is always the
*moving* matmul operand at the fp8 free-dim maximum (rhs [128,2,512],
DoubleRow) while the skinny operand is stationary:
    pass 1:  w^T[c, m]  = h2^T A^T  (+ h1^T)      c = 128 output cols
    pass 2: out^T[c, n]  = w^T  A^T  (+ g0^T)
This doubles per-matmul MACs vs keeping A stationary and moves the
LDWEIGHTS cost to the small reused operand. The zeroth-order term
g0 = x0(Th0-Th2) is folded into the pass-2 PSUM group as one extra
block-diagonal bf16 matmul (lhsT = kron(I4, 65536*(Th0-Th2)), rhs =
x0^T), and the bias rides the final activation's per-partition bias
port - so each output bank drains as a single relu-activation + DMA.

Sharding: 8 cores = 2 batches x (2 t-halves x 2 n-halves). A core's
c-columns are its 4 time steps x 32 output features; its n rows are its
2048-row output block. Pass 1 computes w^T for ALL 4096 m (2x
replicated within a batch across the n-halves - cheaper than any
collective on this runtime), pass 2 only the core's 2048 n columns.
w^T -> w (pass-2 stationary operand needs m on partitions) is done with
PE transposes via a bf16 identity, pipelined per block.

A^T is stored symmetrically slot-permuted (own n-half first on both
axes) so one resident 16 MiB fp8 tile serves pass 1 (all column blocks,
streamed & consumed in arrival order) and pass 2 (column blocks 0-3 =
own n columns; rows in the same slot order as w's chunks). The DMA
stream order IS the prefetch schedule. The trailing column block is
split into two 256-wide halves so the unavoidable serial chain behind
the last bytes (p1 -> scale/add -> transpose -> p2 -> drain) is half
length. Transposes and pass-2 matmuls of slot s-1 are interleaved
between slot-s piece-gated matmul groups so the strict-FIFO PE queue
always has runnable work while the stream is the pacer.
"""

import sys

if "/opt/trn_rl_repo" not in sys.path:
    sys.path.insert(0, "/opt/trn_rl_repo")

import numpy as np
import ml_dtypes

B, T, N, F, O = 2, 8, 4096, 32, 32
K = 3
NCORES = 8
NHALF = N // 2      # 2048 rows per output shard
C = 128             # output columns per core = 4 t-slices x 32 o
NCB = 7             # full-width A^T column blocks of 512 (+ tapered tail)
CBW = 512
NPIECE = 4          # DMA pieces per full column block (512 KiB each)
RCH = 32            # 128-row chunks of A^T
MCH = 32            # 128-row chunks of w

SCALE_A = 4096.0
SCALE_W = 16.0
SCALE_OUT = SCALE_A * SCALE_W

_CACHE = {}


def _build_nc():
    import concourse.mybir as mybir
    import concourse.tile as tile
    from concourse import bacc, masks

    f32 = mybir.dt.float32
    bf16 = mybir.dt.bfloat16
    fp8 = mybir.dt.float8e4
    DR = mybir.MatmulPerfMode.DoubleRow

    nc = bacc.Bacc(None, num_devices=NCORES)

    # all inputs partition-major; A^T symmetrically slot-permuted
    A_d = nc.dram_tensor("A", [NCB, NPIECE, 128, RCH // NPIECE, CBW], fp8,
                         kind="ExternalInput")
    A7_d = nc.dram_tensor("A7", [2, 2, 128, RCH // 2, CBW // 2], fp8,
                          kind="ExternalInput")
    H2_d = nc.dram_tensor("H2", [128, RCH, C], fp8, kind="ExternalInput")
    H1_d = nc.dram_tensor("H1", [128, NCB + 1, CBW], fp8,
                          kind="ExternalInput")
    X0_d = nc.dram_tensor("X0", [128, 4, CBW], bf16, kind="ExternalInput")
    TK_d = nc.dram_tensor("TK", [128, C], bf16, kind="ExternalInput")
    BS_d = nc.dram_tensor("BS", [128, 1], f32, kind="ExternalInput")
    OUT_d = nc.dram_tensor("OUT", [4, 128, CBW], bf16, kind="ExternalOutput")

    with tile.TileContext(nc) as tc:
        with (
            tc.tile_pool(name="big", bufs=1) as big,
            tc.tile_pool(name="stg", bufs=2) as stg,
            tc.tile_pool(name="ps", bufs=1, space="PSUM") as psp,
        ):
            A_sb = big.tile([128, NCB, RCH, CBW], fp8, name="Asb", tag="Asb")
            A7_sb = big.tile([128, 2, RCH, CBW // 2], fp8, name="A7sb",
                             tag="A7sb")
            H2 = big.tile([128, RCH, C], fp8, name="H2s", tag="H2s")
            H1 = big.tile([128, NCB + 1, CBW], fp8, name="H1s", tag="H1s")
            X0 = big.tile([128, 4, CBW], bf16, name="X0s", tag="X0s")
            TK = big.tile([128, C], bf16, name="TKs", tag="TKs")
            BS = big.tile([128, 1], f32, name="BSs", tag="BSs")
            W = big.tile([128, MCH, C], fp8, name="Ws", tag="Ws")
            OS = big.tile([128, 4, CBW], bf16, name="OSs", tag="OSs")
            ident = big.tile([128, 128], bf16, name="ident", tag="ident")

            masks.make_identity(nc, ident[:])

            # ---- one explicitly-ordered input stream on the SP ring ----
            nc.sync.dma_start(H2[:], H2_d[:])
            for pc in range(NPIECE):
                nc.sync.dma_start(
                    A_sb[:, 0, pc * 8:(pc + 1) * 8], A_d[0, pc])
            # side tensors ride the otherwise-idle ACT HWDGE ring so the
            # SP ring stays a pure stream of big transfers
            nc.scalar.dma_start(X0[:], X0_d[:])
            nc.scalar.dma_start(TK[:], TK_d[:])
            nc.scalar.dma_start(BS[:], BS_d[:])
            nc.sync.dma_start(H1[:], H1_d[:])
            for sb in range(1, NCB):
                for pc in range(NPIECE):
                    nc.sync.dma_start(
                        A_sb[:, sb, pc * 8:(pc + 1) * 8], A_d[sb, pc])
            for hh in range(2):
                for pc in range(2):
                    nc.sync.dma_start(
                        A7_sb[:, hh, pc * 16:(pc + 1) * 16], A7_d[hh, pc])

            # ---- PE warm-up (HAM clock gate): dummy matmuls while the
            # first DMAs land so the real matmuls start at 2.4 GHz. Lands
            # in the psum bank pass 1 re-opens with start=True.
            warm_src = big.tile([128, 2, CBW], fp8, name="warmsrc",
                                tag="warmsrc")
            nc.gpsimd.memset(warm_src[:], 0.0)
            warm_ps = psp.tile([128, CBW], f32, name="warm", tag="bank4")
            NWARM = 16
            for wi in range(NWARM):
                nc.tensor.matmul(
                    warm_ps[:, 0:256], warm_src[:, :, 0:128],
                    warm_src[:, :, 0:256],
                    start=(wi == 0), stop=(wi == NWARM - 1), perf_mode=DR)

            # pass-2 psum banks accumulate across the whole stream:
            # 1 bf16 g0 matmul + 16 fp8 DR matmuls each
            ps2 = [psp.tile([128, CBW], f32, name=f"o{nb}", tag=f"bank{nb}")
                   for nb in range(4)]
            p2_count = [0] * 4
            P2N = MCH // 2 + 1

            def p2_mm(mp, nb):
                nc.tensor.matmul(
                    ps2[nb][:],
                    W[:, 2 * mp:2 * mp + 2],
                    A_sb[:, nb, 2 * mp:2 * mp + 2],
                    start=(p2_count[nb] == 0),
                    stop=(p2_count[nb] == P2N - 1),
                    perf_mode=DR)
                p2_count[nb] += 1

            def g0_mm(nb):
                # out^T += 65536 * (Th0-Th2)^T x0^T, block-diagonal in t
                nc.tensor.matmul(
                    ps2[nb][:], TK[:], X0[:, nb],
                    start=(p2_count[nb] == 0),
                    stop=(p2_count[nb] == P2N - 1))
                p2_count[nb] += 1

            def transp(wt, mc, tb):
                # w^T -> w[m, c] 128x128 block via PE (bf16: fp8
                # PE-transpose needs out elem step 2); the psum->W copy
                # casts to fp8 for the pass-2 lhsT
                pst = psp.tile([128, 128], bf16, name=f"t{tb % 2}",
                               tag=f"bank{6 + tb % 2}")
                nc.tensor.transpose(
                    pst[:], wt[:, 128 * tb:128 * (tb + 1)], ident[:])
                nc.vector.tensor_copy(W[:, mc], pst[:])

            Relu = mybir.ActivationFunctionType.Relu

            def drain(nb):
                # out^T = relu(psum/SCALE_OUT + bias_c); DMA on the (now
                # idle) SP ring so the Scalar queue only runs the ACTs
                nc.scalar.activation(OS[:, nb], ps2[nb][:], Relu,
                                     bias=BS[:], scale=1.0 / SCALE_OUT)
                nc.sync.dma_start(OUT_d[nb], OS[:, nb])

            with nc.named_scope("main"):
                # p2q: pass-2 matmuls (and g0 matmuls) whose inputs are
                # safely resident; drained at slot BOUNDARIES - emitted
                # ahead of the next piece-gated p1 group so the strict-
                # FIFO PE queue has runnable work while waiting for the
                # stream - and in the back half of each slot. Transposes
                # of slot s-1 run between the early piece groups of slot
                # s (their STT lands during piece 0).
                p2q = []

                def drainq(n):
                    nonlocal p2q
                    for op in p2q[:n]:
                        op()
                    p2q = p2q[n:]

                prev = None     # (wt, s) of previous slot
                for s in range(NCB):
                    drainq(5)
                    ps1 = psp.tile([128, CBW], f32, name=f"y{s % 2}",
                                   tag=f"bank{4 + s % 2}")

                    def p1_piece(pc):
                        for lp in range(4 * pc, 4 * pc + 4):
                            nc.tensor.matmul(
                                ps1[:], H2[:, 2 * lp:2 * lp + 2],
                                A_sb[:, s, 2 * lp:2 * lp + 2],
                                start=(lp == 0), stop=(lp == RCH // 2 - 1),
                                perf_mode=DR)

                    p1_piece(0)
                    if prev is not None:
                        transp(prev[0], 4 * prev[1] + 0, 0)
                        transp(prev[0], 4 * prev[1] + 1, 1)
                    p1_piece(1)
                    if prev is not None:
                        transp(prev[0], 4 * prev[1] + 2, 2)
                        transp(prev[0], 4 * prev[1] + 3, 3)
                    p1_piece(2)
                    if prev is not None:
                        ps_ = prev[1]
                        for mp in range(2 * ps_, 2 * ps_ + 2):
                            for nb in range(min(ps_ + 1, 4)):
                                p2q.append(
                                    lambda mp=mp, nb=nb: p2_mm(mp, nb))
                        if ps_ < 4:
                            p2q.append(lambda nb=ps_: g0_mm(nb))
                            for mp in range(2 * ps_):
                                p2q.append(
                                    lambda mp=mp, nb=ps_: p2_mm(mp, nb))
                    drainq(3)
                    p1_piece(3)
                    drainq(3)
                    # w^T = psum*(16/4096) + h1^T*16   [bf16 staging]
                    wt = stg.tile([128, CBW], bf16, name="wt", tag="wt")
                    nc.vector.scalar_tensor_tensor(
                        wt[:], ps1[:], 1.0 / 256.0, H1[:, s],
                        mybir.AluOpType.mult, mybir.AluOpType.add)
                    prev = (wt, s)

                # tapered tail: two 256-wide column halves, each its own
                # psum group in one bank, so the post-stream serial chain
                # (p1 -> stt -> transpose -> p2 -> drain) is half length
                drainq(4)
                ps7 = psp.tile([128, CBW], f32, name="y7", tag="bank5")
                for hh in range(2):
                    half = slice(256 * hh, 256 * (hh + 1))
                    for pc in range(2):
                        for lp in range(8 * pc, 8 * pc + 8):
                            nc.tensor.matmul(
                                ps7[:, half], H2[:, 2 * lp:2 * lp + 2],
                                A7_sb[:, hh, 2 * lp:2 * lp + 2],
                                start=(lp == 0), stop=(lp == RCH // 2 - 1),
                                perf_mode=DR)
                        if hh == 0 and pc == 0:
                            transp(prev[0], 4 * prev[1] + 0, 0)
                            transp(prev[0], 4 * prev[1] + 1, 1)
                        elif hh == 0 and pc == 1:
                            transp(prev[0], 4 * prev[1] + 2, 2)
                            transp(prev[0], 4 * prev[1] + 3, 3)
                            ps_ = prev[1]
                            for mp in range(2 * ps_, 2 * ps_ + 2):
                                for nb in range(4):
                                    p2q.append(
                                        lambda mp=mp, nb=nb: p2_mm(mp, nb))
                        else:
                            drainq(4)
                    wt7 = stg.tile([128, 256], bf16, name="wt7", tag="wt7")
                    nc.vector.scalar_tensor_tensor(
                        wt7[:], ps7[:, half], 1.0 / 256.0,
                        H1[:, NCB, half],
                        mybir.AluOpType.mult, mybir.AluOpType.add)
                    if hh == 0:
                        drainq(len(p2q))
                    for tb in range(2):
                        transp(wt7, 28 + 2 * hh + tb, tb)
                    mp = 14 + hh
                    if hh == 0:
                        for nb in range(4):
                            p2_mm(mp, nb)
                    else:
                        for nb in range(4):
                            p2_mm(mp, nb)
                            drain(nb)

    nc.compile()
    return nc


def _get_nc():
    if "nc" not in _CACHE:
        _CACHE["nc"] = _build_nc()
    return _CACHE["nc"]


def _prepare_in_maps(X, A_q, Theta1, bias):
    fp8 = ml_dtypes.float8_e4m3
    bf16 = ml_dtypes.bfloat16
    X = np.asarray(X, dtype=np.float32)
    A_q = np.asarray(A_q, dtype=np.float32)
    Theta1 = np.asarray(Theta1, dtype=np.float32)
    bias = np.asarray(bias, dtype=np.float32)

    Th = Theta1.reshape(F, K, O)
    Th0, Th1, Th2 = Th[:, 0], Th[:, 1], Th[:, 2]

    # 4 unique permuted A^T tile pairs (batch x n-half), shared by t-halves
    A_tiles = {}
    for b in range(B):
        At = (A_q[b].T * SCALE_A).astype(fp8)        # [l/m, m/n]
        for h in range(2):
            if h == 1:
                Ats = np.empty_like(At)
                Ats[:NHALF, :NHALF] = At[NHALF:, NHALF:]
                Ats[:NHALF, NHALF:] = At[NHALF:, :NHALF]
                Ats[NHALF:, :NHALF] = At[:NHALF, NHALF:]
                Ats[NHALF:, NHALF:] = At[:NHALF, :NHALF]
            else:
                Ats = At
            # full blocks: [row, col] -> [cb, piece, p, rc', q]
            main = np.ascontiguousarray(
                Ats[:, :NCB * CBW].reshape(RCH, 128, NCB, CBW)
                .transpose(2, 0, 1, 3)
                .reshape(NCB, NPIECE, RCH // NPIECE, 128, CBW)
                .transpose(0, 1, 3, 2, 4))
            # tapered tail block: two 256-col halves, 2 row pieces each
            tail = np.ascontiguousarray(
                Ats[:, NCB * CBW:].reshape(RCH, 128, 2, CBW // 2)
                .transpose(2, 0, 1, 3)
                .reshape(2, 2, RCH // 2, 128, CBW // 2)
                .transpose(0, 1, 3, 2, 4))
            A_tiles[b, h] = (main, tail)

    in_maps = []
    for core in range(NCORES):
        b, th, h = core // 4, (core // 2) % 2, core % 2
        Xb = X[b, 4 * th:4 * th + 4]                   # (4, N, F)
        sig = np.r_[np.arange(NHALF * h, NHALF * (h + 1)),
                    np.arange(0, NHALF * h), np.arange(NHALF * (h + 1), N)]
        # skinny mats, c = 32*t_rel + o on the trailing axis -> (N, 128)
        h2 = np.transpose(2.0 * (Xb @ Th2), (1, 0, 2)).reshape(N, C)[sig]
        h1 = np.transpose(Xb @ Th1, (1, 0, 2)).reshape(N, C)[sig]
        x0 = Xb[:, NHALF * h:NHALF * (h + 1)]          # (4, 2048, 32)
        main, tail = A_tiles[b, h]
        in_maps.append({
            "A": main,
            "A7": tail,
            "H2": np.ascontiguousarray(
                h2.reshape(RCH, 128, C).transpose(1, 0, 2)).astype(fp8),
            "H1": np.ascontiguousarray(
                (SCALE_W * h1).reshape(NCB + 1, CBW, C)
                .transpose(2, 0, 1)).astype(fp8),
            # x0^T with (t', f) on partitions, own n-half on free
            "X0": np.ascontiguousarray(
                x0.transpose(0, 2, 1).reshape(C, 4, CBW)).astype(bf16),
            "TK": np.ascontiguousarray(
                np.kron(np.eye(4, dtype=np.float32),
                        SCALE_OUT * (Th0 - Th2))).astype(bf16),
            "BS": np.ascontiguousarray(
                np.tile(bias, 4)[:, np.newaxis]).astype(np.float32),
        })
    return in_maps


def run_with_results(inputs, **spmd_kwargs):
    """Returns (full_output, BassKernelResults). spmd_kwargs forwarded to
    run_bass_kernel_spmd (e.g. trace=True)."""
    from concourse.bass_utils import run_bass_kernel_spmd

    nc = _get_nc()
    in_maps = _prepare_in_maps(**inputs)
    res = run_bass_kernel_spmd(
        nc, in_maps, core_ids=list(range(NCORES)), **spmd_kwargs)

    out = np.empty((B, T, N, O), dtype=np.float32)
    for core in range(NCORES):
        b, th, h = core // 4, (core // 2) % 2, core % 2
        blk = np.asarray(res.results[core]["OUT"], dtype=np.float32)
        # [nb, p, q] -> [p, nb*q] -> [t_rel, o, n_local] -> [t, n, o]
        arr = blk.transpose(1, 0, 2).reshape(4, O, NHALF)
        out[b, 4 * th:4 * th + 4, NHALF * h:NHALF * (h + 1), :] = (
            arr.transpose(0, 2, 1))
    return out, res


def kernel(X, A_q, Theta1, bias):
    out, _ = run_with_results(
        {"X": X, "A_q": A_q, "Theta1": Theta1, "bias": bias})
    return out
